# revision 4
# baseline (speedup 1.0000x reference)
"""GSN message-passing kernel for 8 Trainium2 NeuronCores (raw Bass).

Math per layer (algebraically reduced from the reference):
    h   = MLP(x)                       # 3->16->16->3, LeakyReLU(0.01)
    g   = dinv * h                     # dinv = rsqrt(in-degree), 0 if deg==0
    s   = segment_sum(g[row], col)     # the only sparse op
    out = x + dinv*s - (dinv*t)*h      # t[w] = sum_{e:col=w} dinv[row_e]

deg, dinv, t depend only on edge_index and are host-precomputed.
Sharding: targets (col) range-sharded across the 8 cores; each shard is
relabeled by in-degree desc so the k-th-in-edge gather "planes" are
contiguous prefixes. Per layer each core runs the tiny MLP on the vector
engine (broadcast-dense: one multiply + one add per input channel), the
g-tables are AllGathered, then the planes are gather-accumulated with
128-row indirect DMAs (CCE add) into SBUF and combined. The host
concatenates shard outputs and un-permutes the relabeling.

The device program is emitted as raw Bass (no tile framework) with
manual semaphores; the 2x2955 near-identical gather instructions are
template-cloned with patched byte offsets to keep build time low.
Inputs ship small: x/dinv/c1 as float16, gather indices as 3 bytes
each (unpacked on the DVE), weights inlined in the NEFF, output f16.
"""
import sys
sys.path.insert(0, "/opt/trn_rl_repo")

import numpy as np

import concourse.bass as bass
from concourse import mybir

N_VERTS = 1_000_000
N_CORES = 8
P = 128
NEG = 0.01

SHARD = N_VERTS // N_CORES            # 125000 targets per core
COLS = (SHARD + P - 1) // P           # 977 free columns
SHARD_PAD = COLS * P                  # 125056 incl. dummy targets
TABLE_ROWS = N_CORES * SHARD_PAD      # 1000448
TABLE_PAD = TABLE_ROWS + P            # + zero tail rows for plane padding
CH = 489                              # MLP column chunk (SBUF budget)

dt = mybir.dt.float32
dth = mybir.dt.float16
dti = mybir.dt.int32
Add, Mul, Max, Sub = (mybir.AluOpType.add, mybir.AluOpType.mult,
                      mybir.AluOpType.max, mybir.AluOpType.subtract)

MAX_WAITS = 1
_wfix = [0]


def fix_waits(nc):
    """This container's walrus lowers at most 1 sync-wait per instruction;
    split excess waits onto same-engine NoOps placed just before."""
    for f in nc.m.functions:
        for b in f.blocks:
            out, changed = [], False
            for inst in b.instructions:
                si = getattr(inst, "sync_info", None)
                ow = list(si.on_wait) if si is not None and si.on_wait else []
                if len(ow) > MAX_WAITS:
                    changed = True
                    excess, keep = ow[:-MAX_WAITS], ow[-MAX_WAITS:]
                    for i in range(0, len(excess), MAX_WAITS):
                        _wfix[0] += 1
                        out.append(mybir.InstNoOp(
                            name=f"WFIX-{_wfix[0]}", engine=inst.engine,
                            ins=[], outs=[],
                            sync_info=mybir.SyncInfo(
                                on_wait=excess[i:i + MAX_WAITS], on_update=[])))
                    inst.sync_info = mybir.SyncInfo(
                        on_wait=keep, on_update=list(si.on_update or []))
                out.append(inst)
            if changed:
                b.instructions = out


# ---------------------------------------------------------------- host prep

def _host_prep(edge_index):
    """Vectorized prep: per-shard degree-desc relabel, one counting-style
    edge sort, scatter into gather planes. Returns (cores, ncalls)."""
    row = np.asarray(edge_index[0]).astype(np.int32, copy=False)
    col = np.asarray(edge_index[1]).astype(np.int32, copy=False)

    deg = np.bincount(col, minlength=N_VERTS).astype(np.int32)
    dinv = np.where(deg > 0, 1.0 / np.sqrt(np.maximum(deg, 1e-12)), 0.0)
    dinv = dinv.astype(np.float32)

    # per-shard degree-desc relabel, all shards at once (narrow radix key)
    deg2 = deg.reshape(N_CORES, SHARD)
    kmax = int(deg2.max())
    kdt = np.int8 if kmax <= 127 else (np.int16 if kmax <= 2 ** 15 - 1
                                       else np.int32)
    perm = np.argsort((kmax - deg2).astype(kdt), axis=1,
                      kind="stable")                     # w' slot -> local vert
    ar = np.arange(SHARD, dtype=np.int32)
    inv = np.empty((N_CORES, SHARD), np.int32)           # local vert -> w' slot
    np.put_along_axis(inv, perm, ar[None, :], axis=1)
    deg_sorted = np.take_along_axis(deg2, perm, axis=1)  # [8, SHARD] desc

    # per (core, k): number of targets needing a k-th edge, padded to P calls
    mk = (deg_sorted[:, None, :] >= np.arange(1, kmax + 1, dtype=np.int32)
          [None, :, None]).sum(axis=2)                   # [8, kmax]
    ncalls = ((mk.max(axis=0) + P - 1) // P).astype(np.int64)   # [kmax]
    offs = np.concatenate([[0], np.cumsum(ncalls)])      # plane col offsets
    TOT = int(offs[-1])

    # global counting-style sort by target slot (within-group order is
    # irrelevant: addition commutes), one int32 radix argsort of E keys
    wp = inv.reshape(-1)[col]                            # w' of each edge
    shard_of = col // SHARD
    wg = shard_of * SHARD + wp
    order = np.argsort(wg, kind="stable")
    wg_s = wg[order]
    row_s = row[order]
    first = np.r_[True, wg_s[1:] != wg_s[:-1]]
    idx_all = np.arange(len(wg_s), dtype=np.int32)
    start = np.maximum.accumulate(np.where(first, idx_all, 0))
    rank = idx_all - start                               # k-th edge of target

    # t[w] = sum of dinv over w's sources, via segment-reduce on the sort
    seg = np.add.reduceat(dinv[row_s].astype(np.float64), start[first])
    gperm = (perm + (np.arange(N_CORES, dtype=np.int32) * SHARD)[:, None])
    tg = np.zeros(N_VERTS, np.float32)
    tg[wg_s[first]] = seg.astype(np.float32)             # indexed by slot wg
    t = np.zeros(N_VERTS, np.float32)
    t[gperm.reshape(-1)] = tg                            # back to vertex id

    # scatter each edge into its plane slot: core-major [8, TOT*P]
    srcpos = ((row_s // SHARD) * SHARD_PAD + inv.reshape(-1)[row_s]).astype(
        np.int32)                                        # table row of source
    dst = (wg_s // SHARD).astype(np.int64) * (TOT * P) + offs[rank] * P \
        + (wg_s % SHARD)
    buf = np.full(N_CORES * TOT * P, TABLE_ROWS, dtype=np.int32)
    buf[dst] = srcpos
    buf = buf.reshape(N_CORES, TOT, P)

    # pack gather indices as 3 little-endian bytes: [8, P, TOT*3] uint8
    btc = np.ascontiguousarray(buf.transpose(0, 2, 1))   # [8, P, TOT] i32
    pl3 = np.ascontiguousarray(
        btc.view(np.uint8).reshape(N_CORES, P, TOT, 4)[..., :3]
    ).reshape(N_CORES, P, TOT * 3)

    c1 = dinv * t
    cores = []
    for c in range(N_CORES):
        cores.append({"verts_global": gperm[c], "pl3": pl3[c]})
    return cores, [int(n) for n in ncalls], gperm, dinv, c1


def _pad_layout_all(vals, D):
    """[8, SHARD, D] in w'-order -> [8, P, COLS*D]: slot w'=i*P+p at
    [p, i*D+d] per core."""
    buf = np.zeros((N_CORES, SHARD_PAD, D), dtype=np.float32)
    buf[:, :SHARD] = vals.reshape(N_CORES, SHARD, D)
    return buf.reshape(N_CORES, COLS, P, D).transpose(0, 2, 1, 3).reshape(
        N_CORES, P, COLS * D)


def _pack_wtab(weights):
    """Weight rows as a [1, 70*16] table (rows: a1,a2,a3,b1,b2,b3)."""
    wt = np.zeros((70, 16), np.float32)
    r = 0
    for lname in ("a", "b"):
        for wn, n_in in ((lname + "1", 3), (lname + "2", 16), (lname + "3", 16)):
            w = np.asarray(weights[wn], np.float32)
            wt[r:r + n_in, :w.shape[1]] = w
            r += n_in
    assert r == 70
    return np.ascontiguousarray(wt.reshape(1, 70 * 16))


# ------------------------------------------------------------ device kernel

def _build_kernel(ncalls_per_plane, wtab):
    TOT = sum(ncalls_per_plane)
    NQ = 4
    nc = bass.Bass(num_swdge_queues=NQ)

    xc_in = nc.declare_dram_parameter("xc", [P, COLS * 5], dth, isOutput=False)
    pl_in = nc.declare_dram_parameter("pl", [P, TOT * 3], mybir.dt.uint8,
                                      isOutput=False)
    out_ext = nc.declare_dram_parameter("out", [P, COLS * 3], dth, isOutput=True)
    wt_in = nc.inline_tensor(wtab, name="wtab")

    gsh_b = nc.dram_tensor("gsh_b", [SHARD_PAD, 3], dt, kind="Internal")
    table_b = nc.dram_tensor("table_b", [TABLE_PAD, 3], dt, kind="Internal")

    x = nc.alloc_sbuf_tensor("x", [P, COLS, 3], dt)
    dinv = nc.alloc_sbuf_tensor("dinv", [P, COLS, 1], dt)
    c1 = nc.alloc_sbuf_tensor("c1", [P, COLS, 1], dt)
    h = nc.alloc_sbuf_tensor("h", [P, COLS, 3], dt)
    s = nc.alloc_sbuf_tensor("s", [P, COLS, 3], dt)
    gsh = nc.alloc_sbuf_tensor("gsh", [P, COLS, 3], dt)
    wt = nc.alloc_sbuf_tensor("wt", [P, 70, 16], dt)
    xh = nc.alloc_sbuf_tensor("xh", [P, COLS * 5], dth)
    hid = nc.alloc_sbuf_tensor("hid", [P, CH, 16], dt)
    hid2 = nc.alloc_sbuf_tensor("hid2", [P, CH, 16], dt)
    tmp = nc.alloc_sbuf_tensor("tmp", [P, CH, 16], dt)
    tix = nc.alloc_sbuf_tensor("tix", [P, TOT], dti)
    pb = nc.alloc_sbuf_tensor("pb", [P, TOT, 3], mybir.dt.uint8)
    xout = nc.alloc_sbuf_tensor("xout", [P, COLS * 3], dth)
    ztail = nc.alloc_sbuf_tensor("ztail", [P, 3], dt)

    S_IN = nc.alloc_semaphore("S_IN")
    S_GSH = nc.alloc_semaphore("S_GSH")
    S_CC = nc.alloc_semaphore("S_CC")
    S_DVE = nc.alloc_semaphore("S_DVE")
    S_OUT = nc.alloc_semaphore("S_OUT")
    S_Q = [nc.alloc_semaphore(f"S_Q{q}") for q in range(NQ)]

    # ---- input loads (SP engine) ----
    nc.sync.dma_start(out=xh[:], in_=xc_in[:, :]).then_inc(S_IN, 16)
    nc.sync.dma_start(
        out=pb[:].rearrange("p a b -> p (a b)"),
        in_=pl_in[:, :]).then_inc(S_IN, 16)
    nc.sync.dma_start(
        out=wt[:].rearrange("p a b -> p (a b)"),
        in_=wt_in[0:1, :].to_broadcast([P, 70 * 16])).then_inc(S_IN, 16)

    # ---- DVE prologue ----
    nc.vector.wait_ge(S_IN, 48)
    nc.vector.memset(ztail[:], 0.0).then_inc(S_DVE, 1)    # S_DVE: 1
    nc.vector.tensor_copy(
        out=x[:], in_=xh[:, 0:COLS * 3].rearrange("p (c d) -> p c d", d=3))
    nc.vector.tensor_copy(
        out=dinv[:],
        in_=xh[:, COLS * 3:COLS * 4].rearrange("p (c d) -> p c d", d=1))
    nc.vector.tensor_copy(
        out=c1[:],
        in_=xh[:, COLS * 4:COLS * 5].rearrange("p (c d) -> p c d", d=1))
    # unpack 3-byte gather indices: tix = ((b2*256)+b1)*256+b0, via f32
    tixf = tix[:].bitcast(dt)
    nc.vector.tensor_copy(out=tixf, in_=pb[:, :, 2])
    nc.vector.scalar_tensor_tensor(out=tixf, in0=tixf, scalar=256.0,
                                   in1=pb[:, :, 1], op0=Mul, op1=Add)
    nc.vector.scalar_tensor_tensor(out=tixf, in0=tixf, scalar=256.0,
                                   in1=pb[:, :, 0], op0=Mul, op1=Add)
    nc.vector.tensor_copy(out=tix[:], in_=tixf)
    nc.vector.memset(s[:], 0.0).then_inc(S_DVE, 1)        # S_DVE: 2

    # zero tail row block of the table (read by pad slots)
    nc.sync.wait_ge(S_DVE, 1)
    nc.sync.dma_start(out=table_b[TABLE_ROWS:TABLE_PAD, :],
                      in_=ztail[:]).then_inc(S_IN, 16)

    def dense(src, acc, scratch, r0, cw, n_in, n_out):
        # acc[p,c,j] = sum_i src[p,c,i] * wt[p,r0+i,j]
        for i in range(n_in):
            sb_ = src[:, :cw, i:i + 1].to_broadcast([P, cw, n_out])
            wb_ = wt[:, r0 + i:r0 + i + 1, :n_out].to_broadcast([P, cw, n_out])
            if i == 0:
                nc.vector.tensor_tensor(out=acc[:, :cw, :n_out],
                                        in0=sb_, in1=wb_, op=Mul)
            else:
                nc.vector.tensor_tensor(out=scratch[:, :cw, :n_out],
                                        in0=sb_, in1=wb_, op=Mul)
                nc.vector.tensor_add(acc[:, :cw, :n_out],
                                     acc[:, :cw, :n_out],
                                     scratch[:, :cw, :n_out])

    def leaky(t_, cw, n):
        nc.vector.scalar_tensor_tensor(
            out=t_[:, :cw, :n], in0=t_[:, :cw, :n], scalar=NEG,
            in1=t_[:, :cw, :n], op0=Mul, op1=Max)

    def mlp(r0):
        for c0 in range(0, COLS, CH):
            cw = min(CH, COLS - c0)
            dense(x[:, c0:c0 + cw, :], hid, hid2, r0, cw, 3, 16)
            leaky(hid, cw, 16)
            dense(hid, hid2, tmp, r0 + 3, cw, 16, 16)
            leaky(hid2, cw, 16)
            dense(hid2, h[:, c0:c0 + cw, :], hid, r0 + 19, cw, 16, 3)

    # queue round-robin by global column; completion counts per queue
    qcnt_layer = [(TOT - q + NQ - 1) // NQ for q in range(NQ)]
    gather_marks = []

    for layer in range(2):
        # MLP + g-table (DVE)
        mlp(35 * layer)
        nc.vector.tensor_tensor(
            out=gsh[:], in0=h[:],
            in1=dinv[:, :, 0:1].to_broadcast([P, COLS, 3]),
            op=Mul).then_inc(S_DVE, 1)                    # S_DVE: 3 / 6
        # ship g-shard, AllGather into the table (SP -> Pool)
        nc.sync.wait_ge(S_DVE, 3 + 3 * layer)
        nc.sync.dma_start(
            out=gsh_b[:].rearrange("(i p) d -> p i d", p=P),
            in_=gsh[:]).then_inc(S_GSH, 16)
        nc.gpsimd.wait_ge(S_GSH, 16 * (layer + 1))
        nc.gpsimd.collective_compute(
            "AllGather", mybir.AluOpType.bypass,
            replica_groups=[list(range(N_CORES))],
            ins=[gsh_b[:].opt()],
            outs=[table_b[0:TABLE_ROWS, :].opt()],
        ).then_inc(S_CC, 1)
        # gathers (Pool): wait table ready, s zeroed, tix + tail-row loaded
        nc.gpsimd.wait_ge(S_CC, layer + 1)
        nc.gpsimd.wait_ge(S_DVE, 2 + 3 * layer)
        if layer == 0:
            nc.gpsimd.wait_ge(S_IN, 64)
        templates = []
        for q in range(NQ):
            g = nc.gpsimd.indirect_dma_start(
                out=s[:, q, :], out_offset=None,
                in_=table_b[:],
                in_offset=bass.IndirectOffsetOnAxis(
                    ap=tix[:, q:q + 1], axis=0),
                compute_op=Add)
            g.then_inc(S_Q[q], 16)
            if q:
                g.ins.queue = f"qPoolDynamic{q}"
            g.ins.name = f"GT{layer}-{q}"
            templates.append(g.ins)
        gather_marks.append((templates, layer))
        # combine (DVE) after all gathers complete
        for q in range(NQ):
            nc.vector.wait_ge(S_Q[q], 16 * qcnt_layer[q] * (layer + 1))
        nc.vector.tensor_tensor(
            out=s[:], in0=s[:],
            in1=dinv[:, :, 0:1].to_broadcast([P, COLS, 3]), op=Mul)
        nc.vector.tensor_tensor(
            out=h[:], in0=h[:],
            in1=c1[:, :, 0:1].to_broadcast([P, COLS, 3]), op=Mul)
        nc.vector.tensor_add(x[:], x[:], s[:])
        nc.vector.tensor_tensor(out=x[:], in0=x[:], in1=h[:], op=Sub)
        if layer == 0:
            nc.vector.memset(s[:], 0.0).then_inc(S_DVE, 2)  # S_DVE: 5
        else:
            nc.vector.tensor_copy(
                out=xout[:].rearrange("p (c d) -> p c d", d=3),
                in_=x[:]).then_inc(S_DVE, 1)                # S_DVE: 7

    nc.sync.wait_ge(S_DVE, 7)
    nc.sync.dma_start(out=out_ext[:, :], in_=xout[:]).then_inc(S_OUT, 16)
    nc.sync.wait_ge(S_OUT, 16)
    nc.sync.drain()
    nc.all_engine_barrier()

    # ---- expand gather templates into full per-column instruction series ----
    # plane k occupies global columns [offs[k], offs[k]+n_k); column c of
    # plane k gathers into s[:, c-offs[k], :] reading index tix[:, c].
    col2slot = np.empty(TOT, np.int64)
    off = 0
    for n in ncalls_per_plane:
        col2slot[off:off + n] = np.arange(n)
        off += n

    f = nc.m.functions[0]
    for templates, layer in gather_marks:
        tpl_last = templates[-1]
        for b in f.blocks:
            pos = next((i + 1 for i, inst in enumerate(b.instructions)
                        if inst is tpl_last), None)
            if pos is not None:
                clones = []
                PAP, SI = mybir.PhysicalAccessPattern, mybir.SyncInfo
                for c in range(NQ, TOT):
                    t = templates[c % NQ]
                    ti, to = t.ins[1], t.outs[0]
                    g = mybir.InstDMACopy(
                        name=f"G{layer}-{c}", engine=t.engine, queue=t.queue,
                        mode=t.mode,
                        ins=[t.ins[0],
                             PAP(kind='physical_ap', ap=ti.ap, offset=c,
                                 dtype=ti.dtype, dynamic_ap_info=None,
                                 memref=ti.memref, memsetref=ti.memsetref)],
                        outs=[PAP(kind='physical_ap', ap=to.ap,
                                  offset=3 * int(col2slot[c]), dtype=to.dtype,
                                  dynamic_ap_info=None, memref=to.memref,
                                  memsetref=to.memsetref)],
                        oob_is_err=t.oob_is_err, cce_op=t.cce_op,
                        sync_info=SI(on_wait=[],
                                     on_update=list(t.sync_info.on_update)))
                    clones.append(g)
                b.instructions[pos:pos] = clones
                break
        else:
            raise RuntimeError("gather template block not found")

    return nc


# ----------------------------------------------------------------- entry

def kernel(verts, edge_index, W1_0, W2_0, W3_0, W1_1, W2_1, W3_1):
    verts = np.asarray(verts, dtype=np.float32)
    cores, ncalls, gperm, dinv, c1 = _host_prep(np.asarray(edge_index))

    weights = {"a1": np.asarray(W1_0), "a2": np.asarray(W2_0),
               "a3": np.asarray(W3_0), "b1": np.asarray(W1_1),
               "b2": np.asarray(W2_1), "b3": np.asarray(W3_1)}
    nc = _build_kernel(ncalls, _pack_wtab(weights))
    fix_waits(nc)

    xc_all = np.concatenate(
        [_pad_layout_all(verts[gperm], 3),
         _pad_layout_all(dinv[gperm], 1),
         _pad_layout_all(c1[gperm], 1)], axis=2).astype(np.float16)
    in_maps = [{"xc": xc_all[ci], "pl": c["pl3"]}
               for ci, c in enumerate(cores)]

    from concourse.bass_utils import run_bass_kernel_spmd
    res = run_bass_kernel_spmd(nc, in_maps, core_ids=list(range(N_CORES)))

    out = np.empty((N_VERTS, 3), dtype=np.float32)
    for ci, c in enumerate(cores):
        o = res.results[ci]["out"].reshape(P, COLS, 3).transpose(1, 0, 2)
        out[c["verts_global"]] = o.reshape(SHARD_PAD, 3)[:SHARD]
    return out
